# revision 8
# baseline (speedup 1.0000x reference)
"""DeltaNet forward on 8 Trainium2 NeuronCores (Bass/Tile).

Sequence-sharded: L=4096 -> 8 slices of 512 (8 chunks of 64) per core; both
batches + all heads on every core.  The only cross-core dependency is the
per-(b,h,d) diagonal-state recurrence across chunks, factored into per-core
block composites resolved with one small AllGather per batch.

Math decomposition (verified vs reference, ~2.5e-5 relmax in fp32):
- The reference UT-transform loop equals F = 3*(I+A)^{-1} - 2I with
  A = tril(Kb K^T, -1); (I+A)^{-1} computed exactly via the nilpotent
  Neumann product (I-A)(I+A^2)(I+A^4)(I+A^8)(I+A^16)(I+A^32).
- The inter-chunk state S enters outputs only via its diagonal dS:
  dS_{i+1} = a_i*dS_i + c_i, a_i = 1 - sum_c k*w, c_i = sum_c k*u0
  -> elementwise scan (tensor_tensor_scan), block-composited across cores.
- A_intra = tril(einsum('bchd,bchd->bc', q, k)) keeps only chunk-columns
  c <= global batch index, so o_intra touches b+1 columns per chunk.

Precision: projections use a 3-term bf16 split (hi/lo) with fp32 PSUM
accumulation (~2e-4 relmax end to end); chunk-phase matmuls are plain fp32;
the output projection and final-S matmuls use float32r (insensitive).
"""

import contextlib
import sys

import numpy as np

if "/opt/trn_rl_repo" not in sys.path:
    sys.path.insert(0, "/opt/trn_rl_repo")

import ml_dtypes
import concourse.bass as bass
import concourse.tile as tile
from concourse import bacc, mybir
from concourse.bass_utils import run_bass_kernel_spmd

F32 = mybir.dt.float32
F32R = mybir.dt.float32r
BF16 = mybir.dt.bfloat16

NCORES = 8
B, L, HID, H, KS, C = 2, 4096, 1024, 4, 4, 64
DH = HID // H
EPS = 1e-5
LC = L // NCORES
NCH = LC // C
NPT = HID // 128
NLT = LC // 128


def _r11(x):
    """Round-to-nearest at 11 explicit mantissa bits (device float32r)."""
    x64 = x.astype(np.float64)
    scale = 2.0 ** (np.floor(np.log2(np.abs(x64) + 1e-300)) - 11)
    return (np.round(x64 / scale) * scale).astype(np.float32)


def _split_bf(x):
    hi = x.astype(ml_dtypes.bfloat16)
    lo = (x.astype(np.float64) - hi.astype(np.float64)).astype(ml_dtypes.bfloat16)
    return hi, lo


def build_nc():
    nc = bacc.Bacc("TRN2", target_bir_lowering=False, debug=False,
                   num_devices=NCORES)

    def inp(name, shape, dtype=F32):
        return nc.dram_tensor(name, list(shape), dtype, kind="ExternalInput").ap()

    def outp(name, shape, dtype=F32):
        return nc.dram_tensor(name, list(shape), dtype, kind="ExternalOutput").ap()

    io = {}
    # x^T bf16 hi/lo: [b, k(128), kt, l]
    io["xh"] = inp("xh", (B, 128, NPT, LC), BF16)
    io["xl"] = inp("xl", (B, 128, NPT, LC), BF16)
    io["halo"] = inp("halo", (3, B, NPT, 128, KS - 1))
    for nm in "qkv":
        # weight lhsT tiles: [mt, k(128), kt, m]
        io[f"w{nm}h"] = inp(f"w{nm}h", (NPT, 128, NPT, 128), BF16)
        io[f"w{nm}l"] = inp(f"w{nm}l", (NPT, 128, NPT, 128), BF16)
    io["wbh"] = inp("wbh", (128, NPT, H), BF16)
    io["wbl"] = inp("wbl", (128, NPT, H), BF16)
    io["wo"] = inp("wo", (NPT, 128, NPT, 128), F32R)
    io["conv"] = inp("conv", (128, 3, NPT, KS))
    io["rmsw"] = inp("rmsw", (128, NPT))
    io["bmask"] = inp("bmask", (128, NCORES))
    io["bimask"] = inp("bimask", (128, NCORES))

    io["outT"] = outp("outT", (B, NPT, 128, LC))
    io["sp"] = outp("sp", (B, H, DH, DH))

    io["ident"] = nc.inline_tensor(np.eye(128, dtype=np.float32), name="ident").ap()
    ntril = np.tril(np.full((C, C), -1.0, np.float32), -1)
    io["ntril"] = nc.inline_tensor(np.vstack([ntril, ntril]), name="ntril").ap()
    io["ntriu"] = nc.inline_tensor(np.vstack([ntril.T, ntril.T]), name="ntriu").ap()
    eye64 = np.eye(C, dtype=np.float32)
    io["eyep"] = nc.inline_tensor(np.vstack([eye64, eye64]), name="eyep").ap()
    io["twoeyep"] = nc.inline_tensor(np.vstack([2 * eye64, 2 * eye64]),
                                     name="twoeyep").ap()
    bsel = np.zeros((H, H * 128), np.float32)
    for h in range(H):
        bsel[h, h * 128:(h + 1) * 128] = 1.0
    io["bsel"] = nc.inline_tensor(bsel, name="bsel").ap()

    with tile.TileContext(nc) as tc:
        with contextlib.ExitStack() as ctx:
            build_kernel(ctx, tc, io)
    nc.compile()
    return nc


def build_kernel(ctx, tc, io):
    nc = tc.nc
    AF = mybir.ActivationFunctionType
    OP = mybir.AluOpType
    AX = mybir.AxisListType

    consts = ctx.enter_context(tc.tile_pool(name="consts", bufs=1))
    persist = ctx.enter_context(tc.tile_pool(name="persist", bufs=1))
    wpool = ctx.enter_context(tc.tile_pool(name="wpool", bufs=2))
    work = ctx.enter_context(tc.tile_pool(name="work", bufs=2))
    small = ctx.enter_context(tc.tile_pool(name="small", bufs=4))
    neu = ctx.enter_context(tc.tile_pool(name="neu", bufs=2))
    psb = ctx.enter_context(tc.tile_pool(name="psb", bufs=3, space="PSUM"))
    pss = ctx.enter_context(tc.tile_pool(name="pss", bufs=4, space="PSUM"))
    dram = ctx.enter_context(tc.tile_pool(name="dram", bufs=1, space="DRAM"))

    # ---------------- constants ----------------
    ident = consts.tile([128, 128], F32)
    nc.sync.dma_start(ident[:], io["ident"])
    ones128 = consts.tile([128, 128], F32)
    nc.vector.memset(ones128[:], 1.0)
    ones1 = consts.tile([1, 128], F32)
    nc.vector.memset(ones1[:], 1.0)
    eps_col = consts.tile([128, 1], F32)
    nc.vector.memset(eps_col[:], EPS)
    zero_col = consts.tile([128, 1], F32)
    nc.vector.memset(zero_col[:], 0.0)
    ntril = consts.tile([128, C], F32)
    nc.sync.dma_start(ntril[:], io["ntril"])
    ntriu = consts.tile([128, C], F32)
    nc.sync.dma_start(ntriu[:], io["ntriu"])
    eyep = consts.tile([128, C], F32)
    nc.sync.dma_start(eyep[:], io["eyep"])
    twoeyep = consts.tile([128, C], F32)
    nc.sync.dma_start(twoeyep[:], io["twoeyep"])
    bmask = consts.tile([128, NCORES], F32)
    nc.sync.dma_start(bmask[:], io["bmask"])
    bimask = consts.tile([128, NCORES], F32)
    nc.sync.dma_start(bimask[:], io["bimask"])
    conv = consts.tile([128, 3, NPT, KS], F32)
    nc.sync.dma_start(conv[:], io["conv"])
    rmsw = consts.tile([128, NPT], F32)
    nc.sync.dma_start(rmsw[:], io["rmsw"])
    bsel = consts.tile([H, H * 128], F32)
    nc.sync.dma_start(bsel[:], io["bsel"])
    wbh = consts.tile([128, NPT, H], BF16)
    nc.sync.dma_start(wbh[:], io["wbh"])
    wbl = consts.tile([128, NPT, H], BF16)
    nc.sync.dma_start(wbl[:], io["wbl"])

    for b in range(B):
        # ---------------- phase 1: projections ----------------
        x_sb = {}
        for term in ("h", "l"):
            xt = persist.tile([128, NPT, LC], BF16, tag=f"x{term}", name=f"x{term}")
            nc.sync.dma_start(xt[:], io[f"x{term}"][b])
            x_sb[term] = xt

        qn = [persist.tile([128, LC], F32, tag=f"qn{pt}", name=f"qn{pt}") for pt in range(NPT)]
        kn = [persist.tile([128, LC], F32, tag=f"kn{pt}", name=f"kn{pt}") for pt in range(NPT)]
        vn = [persist.tile([128, LC], F32, tag=f"vn{pt}", name=f"vn{pt}") for pt in range(NPT)]
        dest = {"q": qn, "k": kn, "v": vn}

        for ip, pname in enumerate("qkv"):
            for h in range(H):
                t2s = {}
                for sub in range(2):
                    mt = 2 * h + sub
                    wh = wpool.tile([128, NPT, 128], BF16, tag="wh")
                    nc.sync.dma_start(wh[:], io[f"w{pname}h"][mt])
                    wl = wpool.tile([128, NPT, 128], BF16, tag="wl")
                    nc.sync.dma_start(wl[:], io[f"w{pname}l"][mt])
                    ps = psb.tile([128, LC], F32, tag="ps_big")
                    terms = [(wh, "h"), (wh, "l"), (wl, "h")]
                    for kt in range(NPT):
                        for ti, (wt_, xt_) in enumerate(terms):
                            nc.tensor.matmul(
                                ps[:], wt_[:, kt, :], x_sb[xt_][:, kt, :],
                                start=(kt == 0 and ti == 0),
                                stop=(kt == NPT - 1 and ti == 2))
                    pre = work.tile([128, LC + KS - 1], F32, tag="pre", bufs=2)
                    nc.sync.dma_start(pre[:, 0:KS - 1], io["halo"][ip, b, mt])
                    nc.any.tensor_copy(pre[:, KS - 1:], ps[:])
                    # causal depthwise conv: y[l] = sum_t tap[t]*pre[l+t]
                    acc = work.tile([128, LC], F32, tag="cacc0", bufs=1)
                    nc.vector.tensor_scalar_mul(
                        acc[:], pre[:, 0:LC], conv[:, ip, mt, 0:1])
                    for t in range(1, KS):
                        nacc = work.tile([128, LC], F32, tag=f"cacc{t % 2}",
                                         bufs=1)
                        nc.vector.scalar_tensor_tensor(
                            nacc[:], pre[:, t:t + LC], conv[:, ip, mt, t:t + 1],
                            acc[:], op0=OP.mult, op1=OP.add)
                        acc = nacc
                    t1 = work.tile([128, LC], F32, tag="silu1", bufs=1)
                    nc.scalar.activation(t1[:], acc[:], AF.Silu)
                    if pname == "v":
                        nc.scalar.activation(vn[mt][:], t1[:], AF.Silu)
                    else:
                        t2 = work.tile([128, LC], F32, tag=f"t2_{sub}", bufs=2)
                        nc.scalar.activation(t2[:], t1[:], AF.Silu)
                        t2s[sub] = t2
                if pname == "v":
                    continue
                # l2 norm over the head's 256 dims (2 partition tiles)
                psq = psb.tile([128, LC], F32, tag="ps_big")
                for sub in range(2):
                    sq = work.tile([128, LC], F32, tag="sq", bufs=2)
                    nc.vector.tensor_tensor(sq[:], t2s[sub][:], t2s[sub][:],
                                            op=OP.mult)
                    nc.tensor.matmul(psq[:], ones128[:], sq[:],
                                     start=(sub == 0), stop=(sub == 1))
                invn = work.tile([128, LC], F32, tag="invn", bufs=1)
                nc.scalar.activation(invn[:], psq[:], AF.Sqrt, bias=zero_col[:])
                nc.vector.reciprocal(invn[:], invn[:])
                for sub in range(2):
                    nc.vector.tensor_tensor(dest[pname][2 * h + sub][:],
                                            t2s[sub][:], invn[:], op=OP.mult)

        # ---------------- beta ----------------
        psbeta = psb.tile([H, LC], F32, tag="ps_big")
        terms = [(wbh, "h"), (wbh, "l"), (wbl, "h")]
        for kt in range(NPT):
            for ti, (wt_, xt_) in enumerate(terms):
                nc.tensor.matmul(psbeta[:], wt_[:, kt, :], x_sb[xt_][:, kt, :],
                                 start=(kt == 0 and ti == 0),
                                 stop=(kt == NPT - 1 and ti == 2))
        brow = work.tile([H, LC], F32, tag="brow", bufs=1)
        nc.scalar.activation(brow[:], psbeta[:], AF.Sigmoid)
        bnat = []
        for lt in range(NLT):
            pst = pss.tile([128, C], F32, tag="ps_small")
            nc.tensor.transpose(pst[:, 0:H], brow[:, 128 * lt:128 * (lt + 1)],
                                ident[0:H, 0:H])
            bn = persist.tile([128, H], F32, tag=f"bnat{lt}")
            nc.any.tensor_copy(bn[:], pst[:, 0:H])
            bnat.append(bn)
        bbc = []
        for h in range(H):
            psbc = psb.tile([128, LC], F32, tag="ps_big")
            nc.tensor.matmul(psbc[:], bsel[:, 128 * h:128 * (h + 1)], brow[:],
                             start=True, stop=True)
            bb = persist.tile([128, LC], F32, tag=f"bbc{h}")
            nc.any.tensor_copy(bb[:], psbc[:])
            bbc.append(bb)

        # ---------------- phase 2: chunk machinery ----------------
        kbT = [persist.tile([128, LC], F32, tag=f"kbT{pt}", name=f"kbT{pt}") for pt in range(NPT)]
        for pt in range(NPT):
            nc.vector.tensor_tensor(kbT[pt][:], kn[pt][:], bbc[pt // 2][:],
                                    op=OP.mult)

        # A, A^T per chunk (pair-packed), Neumann inverse -> F, F^T
        FTp = []
        for pr in range(NCH // 2):
            psA = pss.tile([128, C], F32, tag="ps_small")
            psAT = pss.tile([128, C], F32, tag="ps_small")
            for half in range(2):
                ch = 2 * pr + half
                sl = slice(C * ch, C * (ch + 1))
                po = 64 * half
                for pt in range(NPT):
                    nc.tensor.matmul(psA[po:po + 64, :], kbT[pt][:, sl],
                                     kn[pt][:, sl], start=(pt == 0),
                                     stop=(pt == NPT - 1), tile_position=(0, po))
                    nc.tensor.matmul(psAT[po:po + 64, :], kn[pt][:, sl],
                                     kbT[pt][:, sl], start=(pt == 0),
                                     stop=(pt == NPT - 1), tile_position=(0, po))
            M = neu.tile([128, C], F32, tag="M")
            nc.vector.tensor_tensor(M[:], psA[:], ntril[:], op=OP.mult)
            MT = neu.tile([128, C], F32, tag="MT")
            nc.vector.tensor_tensor(MT[:], psAT[:], ntriu[:], op=OP.mult)
            acc = neu.tile([128, C], F32, tag="acc")
            nc.vector.tensor_tensor(acc[:], M[:], eyep[:], op=OP.add)
            accT = neu.tile([128, C], F32, tag="accT")
            nc.vector.tensor_tensor(accT[:], MT[:], eyep[:], op=OP.add)
            P, PT = M, MT
            for rnd in range(5):
                psq2 = pss.tile([128, C], F32, tag="ps_small")
                psqT = pss.tile([128, C], F32, tag="ps_small")
                for half in range(2):
                    po = 64 * half
                    hs = slice(po, po + 64)
                    tp = (po, po)
                    nc.tensor.matmul(psq2[hs, :], PT[hs, :], P[hs, :],
                                     start=True, stop=True, tile_position=tp)
                    nc.tensor.matmul(psqT[hs, :], P[hs, :], PT[hs, :],
                                     start=True, stop=True, tile_position=tp)
                Pn = neu.tile([128, C], F32, tag="P")
                nc.any.tensor_copy(Pn[:], psq2[:])
                PnT = neu.tile([128, C], F32, tag="PTn")
                nc.any.tensor_copy(PnT[:], psqT[:])
                pacc = pss.tile([128, C], F32, tag="ps_small")
                paccT = pss.tile([128, C], F32, tag="ps_small")
                for half in range(2):
                    po = 64 * half
                    hs = slice(po, po + 64)
                    tp = (po, po)
                    nc.tensor.matmul(pacc[hs, :], accT[hs, :], Pn[hs, :],
                                     start=True, stop=True, tile_position=tp)
                    nc.tensor.matmul(paccT[hs, :], Pn[hs, :], accT[hs, :],
                                     start=True, stop=True, tile_position=tp)
                nacc = neu.tile([128, C], F32, tag="acc")
                nc.vector.tensor_tensor(nacc[:], pacc[:], acc[:], op=OP.add)
                naccT = neu.tile([128, C], F32, tag="accT")
                nc.vector.tensor_tensor(naccT[:], paccT[:], accT[:], op=OP.add)
                acc, accT, P, PT = nacc, naccT, Pn, PnT
            FmT = persist.tile([128, C], F32, tag=f"FT{pr}")
            nc.vector.scalar_tensor_tensor(FmT[:], accT[:], 3.0, twoeyep[:],
                                           op0=OP.mult, op1=OP.subtract)
            FTp.append(FmT)

        # natural-layout k (kept for S), transient v; W^T, U0^T
        k_nat = [persist.tile([128, HID], F32, tag=f"knat{lt}", name=f"knat{lt}")
                 for lt in range(NLT)]
        wT = [persist.tile([128, LC], F32,
                   tag=(f"wT{pt}" if pt < 4 else f"bbc{pt - 4}"),
                   name=f"wT{pt}") for pt in range(NPT)]
        u0T_g = [persist.tile([128, 4, LC], F32, tag=f"x{t}", name=f"u0T_{t}")
                 for t in ("h", "l")]
        u0T = [u0T_g[pt // 4][:, pt % 4, :] for pt in range(NPT)]
        for lt in range(NLT):
            lsl = slice(128 * lt, 128 * (lt + 1))
            v_nat = work.tile([128, HID], F32, tag="vnat", bufs=1)
            for pt in range(NPT):
                pstr = pss.tile([128, 128], F32, tag="ps_small")
                nc.tensor.transpose(pstr[:], kn[pt][:, lsl], ident[:])
                nc.any.tensor_copy(k_nat[lt][:, 128 * pt:128 * (pt + 1)], pstr[:])
                pstr2 = pss.tile([128, 128], F32, tag="ps_small")
                nc.tensor.transpose(pstr2[:], vn[pt][:, lsl], ident[:])
                nc.any.tensor_copy(v_nat[:, 128 * pt:128 * (pt + 1)], pstr2[:])
            # block-diagonal beta-scaled F^T per head: rows 0:64 -> even-chunk
            # cols 0:64, rows 64:128 -> odd-chunk cols 64:128, zeros elsewhere,
            # so one K=128 matmul computes both chunks of the L-tile.
            ftb = []
            for h in range(H):
                fbs = small.tile([128, C], F32, tag=f"ftbs{h}", bufs=2,
                                 name=f"ftbs{h}")
                nc.vector.tensor_scalar_mul(fbs[:], FTp[lt][:],
                                            bnat[lt][:, h:h + 1])
                fb2 = small.tile([128, 128], F32, tag=f"ftb{h}", bufs=2,
                                 name=f"ftb{h}")
                nc.vector.memset(fb2[:], 0.0)
                nc.vector.tensor_copy(fb2[0:64, 0:64], fbs[0:64, :])
                nc.vector.tensor_copy(fb2[64:128, 64:128], fbs[64:128, :])
                ftb.append(fb2)
            for pt in range(NPT):
                dsl = slice(128 * pt, 128 * (pt + 1))
                fb = ftb[pt // 2]
                psw = pss.tile([128, 128], F32, tag="ps_small")
                psu = pss.tile([128, 128], F32, tag="ps_small")
                nc.tensor.matmul(psw[:], k_nat[lt][:, dsl], fb[:],
                                 start=True, stop=True)
                nc.tensor.matmul(psu[:], v_nat[:, dsl], fb[:],
                                 start=True, stop=True)
                nc.any.tensor_copy(wT[pt][:, lsl], psw[:])
                nc.any.tensor_copy(u0T[pt][:, lsl], psu[:])

        # a, c coefficients; local scan + block composites
        pack_in = dram.tile([128, 2 * NPT], F32, tag="pack")
        acloc = []
        for pt in range(NPT):
            prod = work.tile([128, LC], F32, tag="prod", bufs=1)
            nc.vector.tensor_tensor(prod[:], kn[pt][:], wT[pt][:], op=OP.mult)
            ared = small.tile([128, NCH], F32, tag="ared")
            nc.vector.tensor_reduce(
                ared[:], prod[:].rearrange("p (n c) -> p n c", n=NCH),
                op=OP.add, axis=AX.X)
            a_loc = small.tile([128, NCH], F32, tag=f"a_loc{pt}", bufs=1)
            nc.vector.tensor_scalar(a_loc[:], ared[:], scalar1=-1.0, scalar2=1.0,
                                    op0=OP.mult, op1=OP.add)
            prod2 = work.tile([128, LC], F32, tag="prod", bufs=1)
            nc.vector.tensor_tensor(prod2[:], kn[pt][:], u0T[pt][:], op=OP.mult)
            c_loc = small.tile([128, NCH], F32, tag=f"c_loc{pt}", bufs=1)
            nc.vector.tensor_reduce(
                c_loc[:], prod2[:].rearrange("p (n c) -> p n c", n=NCH),
                op=OP.add, axis=AX.X)
            loc_incl = small.tile([128, NCH], F32, tag="loc_incl")
            nc.vector.tensor_tensor_scan(loc_incl[:], a_loc[:], c_loc[:], 0.0,
                                         op0=OP.mult, op1=OP.add)
            a_blk = small.tile([128, 1], F32, tag="a_blk")
            nc.vector.tensor_reduce(a_blk[:], a_loc[:], op=OP.mult, axis=AX.X)
            nc.sync.dma_start(pack_in[:, 2 * pt:2 * pt + 1], a_blk[:])
            nc.sync.dma_start(pack_in[:, 2 * pt + 1:2 * pt + 2],
                              loc_incl[:, NCH - 1:NCH])
            acloc.append((a_loc, c_loc))

        pack_out = dram.tile([NCORES * 128, 2 * NPT], F32, tag="packo")
        nc.gpsimd.collective_compute(
            "AllGather", mybir.AluOpType.bypass,
            replica_groups=[list(range(NCORES))],
            ins=[pack_in[:].opt()],
            outs=[pack_out[:].opt()],
        )
        gath = pack_out[:].rearrange("(cr p) t -> p t cr", cr=NCORES)

        ds_cols = []
        for pt in range(NPT):
            a_loc, c_loc = acloc[pt]
            ablk_all = small.tile([128, NCORES], F32, tag="ablk_all")
            nc.sync.dma_start(ablk_all[:], gath[:, 2 * pt, :])
            cblk_all = small.tile([128, NCORES], F32, tag="cblk_all")
            nc.sync.dma_start(cblk_all[:], gath[:, 2 * pt + 1, :])
            aeff = small.tile([128, NCORES], F32, tag="aeff")
            nc.vector.tensor_tensor(aeff[:], ablk_all[:], bmask[:], op=OP.mult)
            aeff2 = small.tile([128, NCORES], F32, tag="aeff2")
            nc.vector.tensor_tensor(aeff2[:], aeff[:], bimask[:], op=OP.add)
            ceff = small.tile([128, NCORES], F32, tag="ceff")
            nc.vector.tensor_tensor(ceff[:], cblk_all[:], bmask[:], op=OP.mult)
            blk_incl = small.tile([128, NCORES], F32, tag="blk_incl")
            nc.vector.tensor_tensor_scan(blk_incl[:], aeff2[:], ceff[:], 0.0,
                                         op0=OP.mult, op1=OP.add)
            incoming = blk_incl[:, NCORES - 1:NCORES]
            loc2 = small.tile([128, NCH], F32, tag="loc2")
            nc.vector.tensor_tensor_scan(loc2[:], a_loc[:], c_loc[:], incoming,
                                         op0=OP.mult, op1=OP.add)
            dsp = small.tile([128, NCH], F32, tag=f"dsp{pt}", bufs=1)
            nc.any.tensor_copy(dsp[:, 0:1], incoming)
            nc.any.tensor_copy(dsp[:, 1:NCH], loc2[:, 0:NCH - 1])
            dsn = small.tile([128, NCH], F32, tag=f"dsn{pt}", bufs=1)
            nc.vector.tensor_scalar_mul(dsn[:], dsp[:], -1.0)
            ds_cols.append((dsp, dsn))

        # intra-chunk attention coefficients (columns c <= b of each chunk)
        ncols = b + 1
        psar = pss.tile([128, NCH * ncols], F32, tag="ps_small")
        for pt in range(NPT):
            qk = work.tile([128, NCH, ncols], F32, tag="qk", bufs=2)
            cols_q = qn[pt][:].rearrange("p (n c) -> p n c", n=NCH)[:, :, 0:ncols]
            cols_k = kn[pt][:].rearrange("p (n c) -> p n c", n=NCH)[:, :, 0:ncols]
            nc.vector.tensor_tensor(qk[:], cols_q, cols_k, op=OP.mult)
            nc.tensor.matmul(psar[:], ones128[:],
                             qk[:].rearrange("p n c -> p (n c)"),
                             start=(pt == 0), stop=(pt == NPT - 1))
        araw = work.tile([128, NCH, ncols], F32, tag="araw", bufs=1)
        nc.any.tensor_copy(araw[:].rearrange("p n c -> p (n c)"), psar[:])

        # u = u0 - w*dS, o = q*dS (+ intra); transpose u -> u_nat per L-tile
        oT = [persist.tile([128, LC], F32, tag=f"vn{pt}", name=f"oT{pt}") for pt in range(NPT)]
        u_nat = [persist.tile([128, HID], F32, tag=f"kbT{lt}", name=f"unat{lt}")
                 for lt in range(NLT)]
        for pt in range(NPT):
            dsp, dsn = ds_cols[pt]
            for j in range(NCH):
                sl = slice(C * j, C * (j + 1))
                nc.vector.tensor_scalar_mul(oT[pt][:, sl], qn[pt][:, sl],
                                            dsp[:, j:j + 1])
            for lt in range(NLT):
                lsl = slice(128 * lt, 128 * (lt + 1))
                utmp = work.tile([128, 128], F32, tag="utmp", bufs=3)
                for jj in range(2):
                    j = 2 * lt + jj
                    sl = slice(C * j, C * (j + 1))
                    nc.vector.scalar_tensor_tensor(
                        utmp[:, C * jj:C * (jj + 1)], wT[pt][:, sl],
                        dsn[:, j:j + 1], u0T[pt][:, sl],
                        op0=OP.mult, op1=OP.add)
                # intra contribution on the first `ncols` columns of each chunk
                for jj in range(2):
                    j = 2 * lt + jj
                    itmp = small.tile([128, ncols], F32, tag="itmp")
                    nc.vector.tensor_tensor(itmp[:],
                                            utmp[:, C * jj:C * jj + ncols],
                                            araw[:, j, :], op=OP.mult)
                    o2 = small.tile([128, ncols], F32, tag="o2")
                    nc.vector.tensor_tensor(
                        o2[:], itmp[:], oT[pt][:, C * j:C * j + ncols],
                        op=OP.add)
                    nc.vector.tensor_copy(oT[pt][:, C * j:C * j + ncols], o2[:])
                pstr = pss.tile([128, 128], F32, tag="ps_small")
                nc.tensor.transpose(pstr[:], utmp[:], ident[:])
                nc.any.tensor_copy(u_nat[lt][:, 128 * pt:128 * (pt + 1)],
                                   pstr[:])

        # RMS norm + output projection (float32r)
        psr = psb.tile([128, LC], F32, tag="ps_big")
        for pt in range(NPT):
            osq = work.tile([128, LC], F32, tag="sq", bufs=2)
            nc.vector.tensor_tensor(osq[:], oT[pt][:], oT[pt][:], op=OP.mult)
            nc.tensor.matmul(psr[:], ones128[:], osq[:],
                             start=(pt == 0), stop=(pt == NPT - 1))
        rms = work.tile([128, LC], F32, tag="rms", bufs=1)
        nc.scalar.activation(rms[:], psr[:], AF.Sqrt, bias=eps_col[:],
                             scale=1.0 / HID)
        nc.vector.reciprocal(rms[:], rms[:])
        onr_g = [persist.tile([128, 4, LC], F32R, tag=f"x{t}", name=f"onr_{t}")
                 for t in ("h", "l")]
        onr = [onr_g[pt // 4][:, pt % 4, :] for pt in range(NPT)]
        for pt in range(NPT):
            nc.vector.scalar_tensor_tensor(onr[pt][:], oT[pt][:],
                                           rmsw[:, pt:pt + 1], rms[:],
                                           op0=OP.mult, op1=OP.mult)
        for mt in range(NPT):
            wo = wpool.tile([128, NPT, 128], F32R, tag="wo")
            nc.sync.dma_start(wo[:], io["wo"][mt])
            pso = psb.tile([128, LC], F32, tag="ps_big")
            for kt in range(NPT):
                nc.tensor.matmul(pso[:], wo[:, kt, :], onr[kt][:],
                                 start=(kt == 0), stop=(kt == NPT - 1))
            osb = work.tile([128, LC], F32, tag="osb", bufs=2)
            nc.any.tensor_copy(osb[:], pso[:])
            nc.sync.dma_start(io["outT"][b, mt], osb[:])

        # partial S: S[d, e] = sum_l k_nat[l, d] * u_nat[l, e]
        for h in range(H):
            for d2 in range(2):
                dsl = slice(DH * h + 128 * d2, DH * h + 128 * (d2 + 1))
                esl = slice(DH * h, DH * (h + 1))
                psS = psb.tile([128, DH], F32, tag="ps_big")
                for lt in range(NLT):
                    nc.tensor.matmul(psS[:], k_nat[lt][:, dsl],
                                     u_nat[lt][:, esl],
                                     start=(lt == 0), stop=(lt == NLT - 1))
                ssb = work.tile([128, DH], F32, tag="ssb", bufs=2)
                nc.any.tensor_copy(ssb[:], psS[:])
                nc.sync.dma_start(io["sp"][b, h, 128 * d2:128 * (d2 + 1), :],
                                  ssb[:])


_NC_CACHE = None


def _build_in_maps(inputs):
    x = np.asarray(inputs["x"], np.float32)
    Wq = np.asarray(inputs["Wq"], np.float32)
    Wk = np.asarray(inputs["Wk"], np.float32)
    Wv = np.asarray(inputs["Wv"], np.float32)
    conv_q = np.asarray(inputs["conv_q"], np.float32)
    conv_k = np.asarray(inputs["conv_k"], np.float32)
    conv_v = np.asarray(inputs["conv_v"], np.float32)
    Wbeta = np.asarray(inputs["Wbeta"], np.float32)
    rms_w = np.asarray(inputs["rms_w"], np.float32)
    Wo = np.asarray(inputs["Wo"], np.float32)

    def prep_w(W, split=True):
        # lhsT tiles [mt, k(128), kt, m] from W^T (hid, out)
        wt = np.ascontiguousarray(W.T)
        tiles = wt.reshape(NPT, 128, NPT, 128).transpose(2, 1, 0, 3)
        if split:
            return _split_bf(np.ascontiguousarray(tiles))
        return np.ascontiguousarray(tiles)

    wqh, wql = prep_w(Wq)
    wkh, wkl = prep_w(Wk)
    wvh, wvl = prep_w(Wv)
    wo = _r11(prep_w(Wo, split=False))
    wbt = np.ascontiguousarray(Wbeta.T).reshape(NPT, 128, H).transpose(1, 0, 2)
    wbh, wbl = _split_bf(np.ascontiguousarray(wbt))
    conv = np.stack([
        conv_q[:, 0, :].reshape(NPT, 128, KS),
        conv_k[:, 0, :].reshape(NPT, 128, KS),
        conv_v[:, 0, :].reshape(NPT, 128, KS),
    ]).transpose(2, 0, 1, 3)                                  # (128, 3, NPT, KS)
    conv = np.ascontiguousarray(conv, np.float32)
    rmsw = np.ascontiguousarray(rms_w.reshape(NPT, 128).T, np.float32)

    in_maps = []
    for r in range(NCORES):
        lo_pos = LC * r
        xs = x[:, lo_pos:lo_pos + LC, :]
        xT = np.ascontiguousarray(xs.transpose(0, 2, 1))      # (B, HID, LC)
        xT = xT.reshape(B, NPT, 128, LC).transpose(0, 2, 1, 3)  # (B,128,NPT,LC)
        xh, xl = _split_bf(np.ascontiguousarray(xT))
        if r == 0:
            xhalo = np.zeros((B, KS - 1, HID), np.float32)
        else:
            xhalo = x[:, lo_pos - (KS - 1):lo_pos, :]
        halo = np.stack([
            np.einsum("bth,oh->bot", xhalo, W).astype(np.float32)
            for W in (Wq, Wk, Wv)
        ])                                                    # (3, B, HID, 3)
        halo = np.ascontiguousarray(halo.reshape(3, B, NPT, 128, KS - 1))
        bm = np.zeros((128, NCORES), np.float32)
        bm[:, :r] = 1.0
        in_maps.append({
            "xh": xh, "xl": xl, "halo": halo,
            "wqh": wqh, "wql": wql, "wkh": wkh, "wkl": wkl,
            "wvh": wvh, "wvl": wvl, "wbh": wbh, "wbl": wbl, "wo": wo,
            "conv": conv, "rmsw": rmsw,
            "bmask": bm, "bimask": np.ascontiguousarray(1.0 - bm),
        })
    return in_maps


def kernel(**inputs):
    global _NC_CACHE
    in_maps = _build_in_maps(inputs)
    if _NC_CACHE is None:
        _NC_CACHE = build_nc()
    res = run_bass_kernel_spmd(_NC_CACHE, in_maps, core_ids=list(range(NCORES)))

    outs = []
    S = np.zeros((B, H, DH, DH), np.float32)
    for r in range(NCORES):
        om = res.results[r]
        outs.append(om["outT"].reshape(B, HID, LC).transpose(0, 2, 1))
        S += om["sp"]
    out = np.ascontiguousarray(np.concatenate(outs, axis=1), np.float32)
    return out, S


# revision 10
# speedup vs baseline: 1.0431x; 1.0431x over previous
"""DeltaNet forward on 8 Trainium2 NeuronCores (Bass/Tile).

Sequence-sharded: L=4096 -> 8 slices of 512 (8 chunks of 64) per core; both
batches + all heads on every core.  The only cross-core dependency is the
per-(b,h,d) diagonal-state recurrence across chunks, factored into per-core
block composites resolved with one small AllGather per batch.

Math decomposition (verified vs reference, ~2.5e-5 relmax in fp32):
- The reference UT-transform loop equals F = 3*(I+A)^{-1} - 2I with
  A = tril(Kb K^T, -1); (I+A)^{-1} computed exactly via the nilpotent
  Neumann product (I-A)(I+A^2)(I+A^4)(I+A^8)(I+A^16)(I+A^32).
- The inter-chunk state S enters outputs only via its diagonal dS:
  dS_{i+1} = a_i*dS_i + c_i, a_i = 1 - sum_c k*w, c_i = sum_c k*u0
  -> elementwise scan (tensor_tensor_scan), block-composited across cores.
- A_intra = tril(einsum('bchd,bchd->bc', q, k)) keeps only chunk-columns
  c <= global batch index, so o_intra touches b+1 columns per chunk.

Precision: projections use a 3-term bf16 split (hi/lo) with fp32 PSUM
accumulation (~2e-4 relmax end to end); chunk-phase matmuls are plain fp32;
the output projection and final-S matmuls use float32r (insensitive).
"""

import contextlib
import sys

import numpy as np

if "/opt/trn_rl_repo" not in sys.path:
    sys.path.insert(0, "/opt/trn_rl_repo")

import ml_dtypes
import concourse.bass as bass
import concourse.tile as tile
from concourse import bacc, mybir
from concourse.bass_utils import run_bass_kernel_spmd

F32 = mybir.dt.float32
F32R = mybir.dt.float32r
BF16 = mybir.dt.bfloat16

NCORES = 8
B, L, HID, H, KS, C = 2, 4096, 1024, 4, 4, 64
DH = HID // H
EPS = 1e-5
LC = L // NCORES
NCH = LC // C
NPT = HID // 128
NLT = LC // 128


def _r11(x):
    """Round-to-nearest at 11 explicit mantissa bits (device float32r)."""
    x64 = x.astype(np.float64)
    scale = 2.0 ** (np.floor(np.log2(np.abs(x64) + 1e-300)) - 11)
    return (np.round(x64 / scale) * scale).astype(np.float32)


def _split_bf(x):
    hi = x.astype(ml_dtypes.bfloat16)
    lo = (x.astype(np.float64) - hi.astype(np.float64)).astype(ml_dtypes.bfloat16)
    return hi, lo


def build_nc():
    nc = bacc.Bacc("TRN2", target_bir_lowering=False, debug=False,
                   num_devices=NCORES)

    def inp(name, shape, dtype=F32):
        return nc.dram_tensor(name, list(shape), dtype, kind="ExternalInput").ap()

    def outp(name, shape, dtype=F32):
        return nc.dram_tensor(name, list(shape), dtype, kind="ExternalOutput").ap()

    io = {}
    # x^T bf16 hi/lo: [b, k(128), kt, l]
    io["xh"] = inp("xh", (B, 128, NPT, LC), BF16)
    io["xl"] = inp("xl", (B, 128, NPT, LC), BF16)
    io["halo"] = inp("halo", (3, B, NPT, 128, KS - 1))
    for nm in "qkv":
        # weight lhsT tiles: [mt, k(128), kt, m]
        io[f"w{nm}h"] = inp(f"w{nm}h", (NPT, 128, NPT, 128), BF16)
        io[f"w{nm}l"] = inp(f"w{nm}l", (NPT, 128, NPT, 128), BF16)
    io["wbh"] = inp("wbh", (128, NPT, H), BF16)
    io["wbl"] = inp("wbl", (128, NPT, H), BF16)
    io["wo"] = inp("wo", (NPT, 128, NPT, 128), F32R)
    io["conv"] = inp("conv", (128, 3, NPT, KS))
    io["rmsw"] = inp("rmsw", (128, NPT))
    io["bmask"] = inp("bmask", (128, NCORES))
    io["bimask"] = inp("bimask", (128, NCORES))

    io["outT"] = outp("outT", (B, NPT, 128, LC))
    io["sp"] = outp("sp", (B, H, DH, DH))

    io["ident"] = nc.inline_tensor(np.eye(128, dtype=np.float32), name="ident").ap()
    ntril = np.tril(np.full((C, C), -1.0, np.float32), -1)
    io["ntril"] = nc.inline_tensor(np.vstack([ntril, ntril]), name="ntril").ap()
    io["ntriu"] = nc.inline_tensor(np.vstack([ntril.T, ntril.T]), name="ntriu").ap()
    eye64 = np.eye(C, dtype=np.float32)
    io["eyep"] = nc.inline_tensor(np.vstack([eye64, eye64]), name="eyep").ap()
    io["twoeyep"] = nc.inline_tensor(np.vstack([2 * eye64, 2 * eye64]),
                                     name="twoeyep").ap()
    bsel = np.zeros((H, H * 128), np.float32)
    for h in range(H):
        bsel[h, h * 128:(h + 1) * 128] = 1.0
    io["bsel"] = nc.inline_tensor(bsel, name="bsel").ap()

    with tile.TileContext(nc) as tc:
        with contextlib.ExitStack() as ctx:
            build_kernel(ctx, tc, io)
    nc.compile()
    return nc


def build_kernel(ctx, tc, io):
    nc = tc.nc
    AF = mybir.ActivationFunctionType
    OP = mybir.AluOpType
    AX = mybir.AxisListType

    consts = ctx.enter_context(tc.tile_pool(name="consts", bufs=1))
    persist = ctx.enter_context(tc.tile_pool(name="persist", bufs=1))
    wpool = ctx.enter_context(tc.tile_pool(name="wpool", bufs=2))
    work = ctx.enter_context(tc.tile_pool(name="work", bufs=2))
    small = ctx.enter_context(tc.tile_pool(name="small", bufs=4))
    neu = ctx.enter_context(tc.tile_pool(name="neu", bufs=2))
    psb = ctx.enter_context(tc.tile_pool(name="psb", bufs=3, space="PSUM"))
    pss = ctx.enter_context(tc.tile_pool(name="pss", bufs=4, space="PSUM"))
    dram = ctx.enter_context(tc.tile_pool(name="dram", bufs=1, space="DRAM"))

    # ---------------- constants ----------------
    ident = consts.tile([128, 128], F32)
    nc.sync.dma_start(ident[:], io["ident"])
    ones128 = consts.tile([128, 128], F32)
    nc.vector.memset(ones128[:], 1.0)
    ones128r = consts.tile([128, 128], F32R)
    nc.vector.tensor_copy(ones128r[:], ones128[:])
    ones1 = consts.tile([1, 128], F32)
    nc.vector.memset(ones1[:], 1.0)
    eps_col = consts.tile([128, 1], F32)
    nc.vector.memset(eps_col[:], EPS)
    zero_col = consts.tile([128, 1], F32)
    nc.vector.memset(zero_col[:], 0.0)
    ntril = consts.tile([128, C], F32)
    nc.sync.dma_start(ntril[:], io["ntril"])
    ntriu = consts.tile([128, C], F32)
    nc.sync.dma_start(ntriu[:], io["ntriu"])
    eyep = consts.tile([128, C], F32)
    nc.sync.dma_start(eyep[:], io["eyep"])
    twoeyep = consts.tile([128, C], F32)
    nc.sync.dma_start(twoeyep[:], io["twoeyep"])
    bmask = consts.tile([128, NCORES], F32)
    nc.sync.dma_start(bmask[:], io["bmask"])
    bimask = consts.tile([128, NCORES], F32)
    nc.sync.dma_start(bimask[:], io["bimask"])
    conv = consts.tile([128, 3, NPT, KS], F32)
    nc.sync.dma_start(conv[:], io["conv"])
    rmsw = consts.tile([128, NPT], F32)
    nc.sync.dma_start(rmsw[:], io["rmsw"])
    bsel = consts.tile([H, H * 128], F32)
    nc.sync.dma_start(bsel[:], io["bsel"])
    wbh = consts.tile([128, NPT, H], BF16)
    nc.sync.dma_start(wbh[:], io["wbh"])
    wbl = consts.tile([128, NPT, H], BF16)
    nc.sync.dma_start(wbl[:], io["wbl"])

    for b in range(B):
        # ---------------- phase 1: projections ----------------
        x_sb = {}
        for term in ("h", "l"):
            xt = persist.tile([128, NPT, LC], BF16, tag=f"x{term}", name=f"x{term}")
            nc.sync.dma_start(xt[:], io[f"x{term}"][b])
            x_sb[term] = xt

        qn = [persist.tile([128, LC], F32, tag=f"qn{pt}", name=f"qn{pt}") for pt in range(NPT)]
        kn = [persist.tile([128, LC], F32, tag=f"kn{pt}", name=f"kn{pt}") for pt in range(NPT)]
        vn = [persist.tile([128, LC], F32, tag=f"vn{pt}", name=f"vn{pt}") for pt in range(NPT)]
        dest = {"q": qn, "k": kn, "v": vn}

        for ip, pname in enumerate("qkv"):
            for h in range(H):
                t2s = {}
                for sub in range(2):
                    mt = 2 * h + sub
                    wh = wpool.tile([128, NPT, 128], BF16, tag="wh")
                    nc.sync.dma_start(wh[:], io[f"w{pname}h"][mt])
                    wl = wpool.tile([128, NPT, 128], BF16, tag="wl")
                    nc.sync.dma_start(wl[:], io[f"w{pname}l"][mt])
                    ps = psb.tile([128, LC], F32, tag="ps_big")
                    terms = [(wh, "h"), (wh, "l"), (wl, "h")]
                    for kt in range(NPT):
                        for ti, (wt_, xt_) in enumerate(terms):
                            nc.tensor.matmul(
                                ps[:], wt_[:, kt, :], x_sb[xt_][:, kt, :],
                                start=(kt == 0 and ti == 0),
                                stop=(kt == NPT - 1 and ti == 2))
                    pre = work.tile([128, LC + KS - 1], F32, tag="pre", bufs=2)
                    nc.sync.dma_start(pre[:, 0:KS - 1], io["halo"][ip, b, mt])
                    nc.any.tensor_copy(pre[:, KS - 1:], ps[:])
                    # causal depthwise conv: y[l] = sum_t tap[t]*pre[l+t]
                    acc = work.tile([128, LC], F32, tag="cacc0", bufs=1)
                    nc.vector.tensor_scalar_mul(
                        acc[:], pre[:, 0:LC], conv[:, ip, mt, 0:1])
                    for t in range(1, KS):
                        nacc = work.tile([128, LC], F32, tag=f"cacc{t % 2}",
                                         bufs=1)
                        nc.vector.scalar_tensor_tensor(
                            nacc[:], pre[:, t:t + LC], conv[:, ip, mt, t:t + 1],
                            acc[:], op0=OP.mult, op1=OP.add)
                        acc = nacc
                    t1 = work.tile([128, LC], F32, tag="silu1", bufs=1)
                    nc.scalar.activation(t1[:], acc[:], AF.Silu)
                    if pname == "v":
                        nc.scalar.activation(vn[mt][:], t1[:], AF.Silu)
                    else:
                        t2 = work.tile([128, LC], F32, tag=f"t2_{sub}", bufs=2)
                        nc.scalar.activation(t2[:], t1[:], AF.Silu)
                        t2s[sub] = t2
                if pname == "v":
                    continue
                # l2 norm over the head's 256 dims (2 partition tiles)
                psq = psb.tile([128, LC], F32, tag="ps_big")
                for sub in range(2):
                    sq = work.tile([128, LC], F32, tag="sq", bufs=2)
                    nc.gpsimd.tensor_tensor(sq[:], t2s[sub][:], t2s[sub][:],
                                            op=OP.mult)
                    nc.tensor.matmul(psq[:], ones128[:], sq[:],
                                     start=(sub == 0), stop=(sub == 1))
                invn = work.tile([128, LC], F32, tag="invn", bufs=1)
                nc.scalar.activation(invn[:], psq[:], AF.Sqrt, bias=zero_col[:])
                nc.vector.reciprocal(invn[:], invn[:])
                for sub in range(2):
                    nc.vector.tensor_tensor(dest[pname][2 * h + sub][:],
                                            t2s[sub][:], invn[:], op=OP.mult)

        # ---------------- beta ----------------
        psbeta = psb.tile([H, LC], F32, tag="ps_big")
        terms = [(wbh, "h"), (wbh, "l"), (wbl, "h")]
        for kt in range(NPT):
            for ti, (wt_, xt_) in enumerate(terms):
                nc.tensor.matmul(psbeta[:], wt_[:, kt, :], x_sb[xt_][:, kt, :],
                                 start=(kt == 0 and ti == 0),
                                 stop=(kt == NPT - 1 and ti == 2))
        brow = work.tile([H, LC], F32, tag="brow", bufs=1)
        nc.scalar.activation(brow[:], psbeta[:], AF.Sigmoid)
        bnat = []
        for lt in range(NLT):
            pst = pss.tile([128, C], F32, tag="ps_small")
            nc.tensor.transpose(pst[:, 0:H], brow[:, 128 * lt:128 * (lt + 1)],
                                ident[0:H, 0:H])
            bn = persist.tile([128, H], F32, tag=f"bnat{lt}")
            nc.any.tensor_copy(bn[:], pst[:, 0:H])
            bnat.append(bn)
        bbc = []
        for h in range(H):
            psbc = psb.tile([128, LC], F32, tag="ps_big")
            nc.tensor.matmul(psbc[:], bsel[:, 128 * h:128 * (h + 1)], brow[:],
                             start=True, stop=True)
            bb = persist.tile([128, LC], F32, tag=f"bbc{h}")
            nc.any.tensor_copy(bb[:], psbc[:])
            bbc.append(bb)

        # ---------------- phase 2: chunk machinery ----------------
        kbT = [persist.tile([128, LC], F32, tag=f"kbT{pt}", name=f"kbT{pt}") for pt in range(NPT)]
        for pt in range(NPT):
            nc.vector.tensor_tensor(kbT[pt][:], kn[pt][:], bbc[pt // 2][:],
                                    op=OP.mult)

        # A, A^T per chunk (pair-packed), Neumann inverse -> F, F^T
        FTp = []
        for pr in range(NCH // 2):
            psA = pss.tile([128, C], F32, tag="ps_small")
            psAT = pss.tile([128, C], F32, tag="ps_small")
            for half in range(2):
                ch = 2 * pr + half
                sl = slice(C * ch, C * (ch + 1))
                po = 64 * half
                for pt in range(NPT):
                    nc.tensor.matmul(psA[po:po + 64, :], kbT[pt][:, sl],
                                     kn[pt][:, sl], start=(pt == 0),
                                     stop=(pt == NPT - 1), tile_position=(0, po))
                    nc.tensor.matmul(psAT[po:po + 64, :], kn[pt][:, sl],
                                     kbT[pt][:, sl], start=(pt == 0),
                                     stop=(pt == NPT - 1), tile_position=(0, po))
            M = neu.tile([128, C], F32, tag="M")
            nc.vector.tensor_tensor(M[:], psA[:], ntril[:], op=OP.mult)
            MT = neu.tile([128, C], F32, tag="MT")
            nc.vector.tensor_tensor(MT[:], psAT[:], ntriu[:], op=OP.mult)
            acc = neu.tile([128, C], F32, tag="acc")
            nc.vector.tensor_tensor(acc[:], M[:], eyep[:], op=OP.add)
            accT = neu.tile([128, C], F32, tag="accT")
            nc.vector.tensor_tensor(accT[:], MT[:], eyep[:], op=OP.add)
            P, PT = M, MT
            for rnd in range(5):
                psq2 = pss.tile([128, C], F32, tag="ps_small")
                psqT = pss.tile([128, C], F32, tag="ps_small")
                for half in range(2):
                    po = 64 * half
                    hs = slice(po, po + 64)
                    tp = (po, po)
                    nc.tensor.matmul(psq2[hs, :], PT[hs, :], P[hs, :],
                                     start=True, stop=True, tile_position=tp)
                    nc.tensor.matmul(psqT[hs, :], P[hs, :], PT[hs, :],
                                     start=True, stop=True, tile_position=tp)
                Pn = neu.tile([128, C], F32, tag="P")
                nc.any.tensor_copy(Pn[:], psq2[:])
                PnT = neu.tile([128, C], F32, tag="PTn")
                nc.any.tensor_copy(PnT[:], psqT[:])
                pacc = pss.tile([128, C], F32, tag="ps_small")
                paccT = pss.tile([128, C], F32, tag="ps_small")
                for half in range(2):
                    po = 64 * half
                    hs = slice(po, po + 64)
                    tp = (po, po)
                    nc.tensor.matmul(pacc[hs, :], accT[hs, :], Pn[hs, :],
                                     start=True, stop=True, tile_position=tp)
                    nc.tensor.matmul(paccT[hs, :], Pn[hs, :], accT[hs, :],
                                     start=True, stop=True, tile_position=tp)
                nacc = neu.tile([128, C], F32, tag="acc")
                nc.vector.tensor_tensor(nacc[:], pacc[:], acc[:], op=OP.add)
                naccT = neu.tile([128, C], F32, tag="accT")
                nc.vector.tensor_tensor(naccT[:], paccT[:], accT[:], op=OP.add)
                acc, accT, P, PT = nacc, naccT, Pn, PnT
            FmT = persist.tile([128, C], F32, tag=f"FT{pr}")
            nc.vector.scalar_tensor_tensor(FmT[:], accT[:], 3.0, twoeyep[:],
                                           op0=OP.mult, op1=OP.subtract)
            FTp.append(FmT)

        # natural-layout k (kept for S), transient v; W^T, U0^T
        k_nat = [persist.tile([128, HID], F32, tag=f"knat{lt}", name=f"knat{lt}")
                 for lt in range(NLT)]
        wT = [persist.tile([128, LC], F32,
                   tag=(f"wT{pt}" if pt < 4 else f"bbc{pt - 4}"),
                   name=f"wT{pt}") for pt in range(NPT)]
        u0T_g = [persist.tile([128, 4, LC], F32, tag=f"x{t}", name=f"u0T_{t}")
                 for t in ("h", "l")]
        u0T = [u0T_g[pt // 4][:, pt % 4, :] for pt in range(NPT)]
        for lt in range(NLT):
            lsl = slice(128 * lt, 128 * (lt + 1))
            v_nat = work.tile([128, HID], F32, tag="vnat", bufs=1)
            for pt in range(NPT):
                pstr = pss.tile([128, 128], F32, tag="ps_small")
                nc.tensor.transpose(pstr[:], kn[pt][:, lsl], ident[:])
                (nc.vector.tensor_copy if pt % 2 else nc.scalar.copy)(
                    k_nat[lt][:, 128 * pt:128 * (pt + 1)], pstr[:])
                pstr2 = pss.tile([128, 128], F32, tag="ps_small")
                nc.tensor.transpose(pstr2[:], vn[pt][:, lsl], ident[:])
                (nc.scalar.copy if pt % 2 else nc.vector.tensor_copy)(
                    v_nat[:, 128 * pt:128 * (pt + 1)], pstr2[:])
            # block-diagonal beta-scaled F^T per head: rows 0:64 -> even-chunk
            # cols 0:64, rows 64:128 -> odd-chunk cols 64:128, zeros elsewhere,
            # so one K=128 matmul computes both chunks of the L-tile.
            ftb = []
            for h in range(H):
                fbs = small.tile([128, C], F32, tag=f"ftbs{h}", bufs=2,
                                 name=f"ftbs{h}")
                nc.vector.tensor_scalar_mul(fbs[:], FTp[lt][:],
                                            bnat[lt][:, h:h + 1])
                fb2 = small.tile([128, 128], F32, tag=f"ftb{h}", bufs=2,
                                 name=f"ftb{h}")
                nc.vector.memset(fb2[:], 0.0)
                nc.vector.tensor_copy(fb2[0:64, 0:64], fbs[0:64, :])
                nc.vector.tensor_copy(fb2[64:128, 64:128], fbs[64:128, :])
                ftb.append(fb2)
            for pt in range(NPT):
                dsl = slice(128 * pt, 128 * (pt + 1))
                fb = ftb[pt // 2]
                psw = pss.tile([128, 128], F32, tag="ps_small")
                psu = pss.tile([128, 128], F32, tag="ps_small")
                nc.tensor.matmul(psw[:], k_nat[lt][:, dsl], fb[:],
                                 start=True, stop=True)
                nc.tensor.matmul(psu[:], v_nat[:, dsl], fb[:],
                                 start=True, stop=True)
                (nc.vector.tensor_copy if pt % 2 else nc.scalar.copy)(
                    wT[pt][:, lsl], psw[:])
                (nc.scalar.copy if pt % 2 else nc.vector.tensor_copy)(
                    u0T[pt][:, lsl], psu[:])

        # a, c coefficients; local scan + block composites
        pack_in = dram.tile([128, 2 * NPT], F32, tag="pack")
        acloc = []
        for pt in range(NPT):
            prod = work.tile([128, LC], F32, tag="prod", bufs=1)
            nc.gpsimd.tensor_tensor(prod[:], kn[pt][:], wT[pt][:], op=OP.mult)
            ared = small.tile([128, NCH], F32, tag="ared")
            nc.vector.tensor_reduce(
                ared[:], prod[:].rearrange("p (n c) -> p n c", n=NCH),
                op=OP.add, axis=AX.X)
            a_loc = small.tile([128, NCH], F32, tag=f"a_loc{pt}", bufs=1)
            nc.vector.tensor_scalar(a_loc[:], ared[:], scalar1=-1.0, scalar2=1.0,
                                    op0=OP.mult, op1=OP.add)
            prod2 = work.tile([128, LC], F32, tag="prod", bufs=1)
            nc.vector.tensor_tensor(prod2[:], kn[pt][:], u0T[pt][:], op=OP.mult)
            c_loc = small.tile([128, NCH], F32, tag=f"c_loc{pt}", bufs=1)
            nc.vector.tensor_reduce(
                c_loc[:], prod2[:].rearrange("p (n c) -> p n c", n=NCH),
                op=OP.add, axis=AX.X)
            loc_incl = small.tile([128, NCH], F32, tag="loc_incl")
            nc.vector.tensor_tensor_scan(loc_incl[:], a_loc[:], c_loc[:], 0.0,
                                         op0=OP.mult, op1=OP.add)
            a_blk = small.tile([128, 1], F32, tag="a_blk")
            nc.vector.tensor_reduce(a_blk[:], a_loc[:], op=OP.mult, axis=AX.X)
            nc.sync.dma_start(pack_in[:, 2 * pt:2 * pt + 1], a_blk[:])
            nc.sync.dma_start(pack_in[:, 2 * pt + 1:2 * pt + 2],
                              loc_incl[:, NCH - 1:NCH])
            acloc.append((a_loc, c_loc))

        pack_out = dram.tile([NCORES * 128, 2 * NPT], F32, tag="packo")
        nc.gpsimd.collective_compute(
            "AllGather", mybir.AluOpType.bypass,
            replica_groups=[list(range(NCORES))],
            ins=[pack_in[:].opt()],
            outs=[pack_out[:].opt()],
        )
        gath = pack_out[:].rearrange("(cr p) t -> p t cr", cr=NCORES)

        ds_cols = []
        for pt in range(NPT):
            a_loc, c_loc = acloc[pt]
            ablk_all = small.tile([128, NCORES], F32, tag="ablk_all")
            nc.sync.dma_start(ablk_all[:], gath[:, 2 * pt, :])
            cblk_all = small.tile([128, NCORES], F32, tag="cblk_all")
            nc.sync.dma_start(cblk_all[:], gath[:, 2 * pt + 1, :])
            aeff = small.tile([128, NCORES], F32, tag="aeff")
            nc.vector.tensor_tensor(aeff[:], ablk_all[:], bmask[:], op=OP.mult)
            aeff2 = small.tile([128, NCORES], F32, tag="aeff2")
            nc.vector.tensor_tensor(aeff2[:], aeff[:], bimask[:], op=OP.add)
            ceff = small.tile([128, NCORES], F32, tag="ceff")
            nc.vector.tensor_tensor(ceff[:], cblk_all[:], bmask[:], op=OP.mult)
            blk_incl = small.tile([128, NCORES], F32, tag="blk_incl")
            nc.vector.tensor_tensor_scan(blk_incl[:], aeff2[:], ceff[:], 0.0,
                                         op0=OP.mult, op1=OP.add)
            incoming = blk_incl[:, NCORES - 1:NCORES]
            loc2 = small.tile([128, NCH], F32, tag="loc2")
            nc.vector.tensor_tensor_scan(loc2[:], a_loc[:], c_loc[:], incoming,
                                         op0=OP.mult, op1=OP.add)
            dsp = small.tile([128, NCH], F32, tag=f"dsp{pt}", bufs=1)
            nc.any.tensor_copy(dsp[:, 0:1], incoming)
            nc.any.tensor_copy(dsp[:, 1:NCH], loc2[:, 0:NCH - 1])
            dsn = small.tile([128, NCH], F32, tag=f"dsn{pt}", bufs=1)
            nc.vector.tensor_scalar_mul(dsn[:], dsp[:], -1.0)
            ds_cols.append((dsp, dsn))

        # intra-chunk attention coefficients (columns c <= b of each chunk)
        ncols = b + 1
        psar = pss.tile([128, NCH * ncols], F32, tag="ps_small")
        for pt in range(NPT):
            qk = work.tile([128, NCH, ncols], F32, tag="qk", bufs=2)
            cols_q = qn[pt][:].rearrange("p (n c) -> p n c", n=NCH)[:, :, 0:ncols]
            cols_k = kn[pt][:].rearrange("p (n c) -> p n c", n=NCH)[:, :, 0:ncols]
            nc.vector.tensor_tensor(qk[:], cols_q, cols_k, op=OP.mult)
            nc.tensor.matmul(psar[:], ones128[:],
                             qk[:].rearrange("p n c -> p (n c)"),
                             start=(pt == 0), stop=(pt == NPT - 1))
        araw = work.tile([128, NCH, ncols], F32, tag="araw", bufs=1)
        nc.any.tensor_copy(araw[:].rearrange("p n c -> p (n c)"), psar[:])

        # u = u0 - w*dS, o = q*dS (+ intra); transpose u -> u_nat per L-tile
        oT = [persist.tile([128, LC], F32, tag=f"vn{pt}", name=f"oT{pt}") for pt in range(NPT)]
        u_nat = [persist.tile([128, HID], F32, tag=f"kbT{lt}", name=f"unat{lt}")
                 for lt in range(NLT)]
        def ds_bc(t):
            return bass.AP(tensor=t[:].tensor, offset=t[:].offset,
                           ap=[t[:].ap[0], t[:].ap[1], [0, C]])

        for pt in range(NPT):
            dsp, dsn = ds_cols[pt]
            nc.vector.tensor_tensor(
                oT[pt][:].rearrange("p (n c) -> p n c", n=NCH),
                qn[pt][:].rearrange("p (n c) -> p n c", n=NCH),
                ds_bc(dsp), op=OP.mult)
            wds = work.tile([128, LC], F32, tag="prod", bufs=1)
            nc.vector.tensor_tensor(
                wds[:].rearrange("p (n c) -> p n c", n=NCH),
                wT[pt][:].rearrange("p (n c) -> p n c", n=NCH),
                ds_bc(dsn), op=OP.mult)
            for lt in range(NLT):
                lsl = slice(128 * lt, 128 * (lt + 1))
                utmp = work.tile([128, 128], F32, tag="utmp", bufs=3)
                nc.vector.tensor_tensor(utmp[:], wds[:, lsl], u0T[pt][:, lsl],
                                        op=OP.add)
                # intra contribution on the first `ncols` columns of each chunk
                for jj in range(2):
                    j = 2 * lt + jj
                    itmp = small.tile([128, ncols], F32, tag="itmp")
                    nc.vector.tensor_tensor(itmp[:],
                                            utmp[:, C * jj:C * jj + ncols],
                                            araw[:, j, :], op=OP.mult)
                    o2 = small.tile([128, ncols], F32, tag="o2")
                    nc.vector.tensor_tensor(
                        o2[:], itmp[:], oT[pt][:, C * j:C * j + ncols],
                        op=OP.add)
                    nc.vector.tensor_copy(oT[pt][:, C * j:C * j + ncols], o2[:])
                pstr = pss.tile([128, 128], F32, tag="ps_small")
                nc.tensor.transpose(pstr[:], utmp[:], ident[:])
                (nc.vector.tensor_copy if pt % 2 else nc.scalar.copy)(
                    u_nat[lt][:, 128 * pt:128 * (pt + 1)], pstr[:])

        # RMS norm + output projection (float32r)
        psr = psb.tile([128, LC], F32, tag="ps_big")
        for pt in range(NPT):
            osq = work.tile([128, LC], F32R, tag="sq", bufs=2)
            nc.vector.tensor_tensor(osq[:], oT[pt][:], oT[pt][:], op=OP.mult)
            nc.tensor.matmul(psr[:], ones128r[:], osq[:],
                             start=(pt == 0), stop=(pt == NPT - 1))
        rms = work.tile([128, LC], F32, tag="rms", bufs=1)
        nc.scalar.activation(rms[:], psr[:], AF.Sqrt, bias=eps_col[:],
                             scale=1.0 / HID)
        nc.vector.reciprocal(rms[:], rms[:])
        onr_g = [persist.tile([128, 4, LC], F32R, tag=f"x{t}", name=f"onr_{t}")
                 for t in ("h", "l")]
        onr = [onr_g[pt // 4][:, pt % 4, :] for pt in range(NPT)]
        for pt in range(NPT):
            nc.vector.scalar_tensor_tensor(onr[pt][:], oT[pt][:],
                                           rmsw[:, pt:pt + 1], rms[:],
                                           op0=OP.mult, op1=OP.mult)
        for mt in range(NPT):
            wo = wpool.tile([128, NPT, 128], F32R, tag="wo")
            nc.sync.dma_start(wo[:], io["wo"][mt])
            pso = psb.tile([128, LC], F32, tag="ps_big")
            for kt in range(NPT):
                nc.tensor.matmul(pso[:], wo[:, kt, :], onr[kt][:],
                                 start=(kt == 0), stop=(kt == NPT - 1))
            osb = work.tile([128, LC], F32, tag="osb", bufs=2)
            nc.any.tensor_copy(osb[:], pso[:])
            nc.sync.dma_start(io["outT"][b, mt], osb[:])

        # partial S: S[d, e] = sum_l k_nat[l, d] * u_nat[l, e]
        for h in range(H):
            for d2 in range(2):
                dsl = slice(DH * h + 128 * d2, DH * h + 128 * (d2 + 1))
                esl = slice(DH * h, DH * (h + 1))
                psS = psb.tile([128, DH], F32, tag="ps_big")
                for lt in range(NLT):
                    nc.tensor.matmul(psS[:], k_nat[lt][:, dsl],
                                     u_nat[lt][:, esl],
                                     start=(lt == 0), stop=(lt == NLT - 1))
                ssb = work.tile([128, DH], F32, tag="ssb", bufs=2)
                nc.any.tensor_copy(ssb[:], psS[:])
                nc.sync.dma_start(io["sp"][b, h, 128 * d2:128 * (d2 + 1), :],
                                  ssb[:])


_NC_CACHE = None


def _build_in_maps(inputs):
    x = np.asarray(inputs["x"], np.float32)
    Wq = np.asarray(inputs["Wq"], np.float32)
    Wk = np.asarray(inputs["Wk"], np.float32)
    Wv = np.asarray(inputs["Wv"], np.float32)
    conv_q = np.asarray(inputs["conv_q"], np.float32)
    conv_k = np.asarray(inputs["conv_k"], np.float32)
    conv_v = np.asarray(inputs["conv_v"], np.float32)
    Wbeta = np.asarray(inputs["Wbeta"], np.float32)
    rms_w = np.asarray(inputs["rms_w"], np.float32)
    Wo = np.asarray(inputs["Wo"], np.float32)

    def prep_w(W, split=True):
        # lhsT tiles [mt, k(128), kt, m] from W^T (hid, out)
        wt = np.ascontiguousarray(W.T)
        tiles = wt.reshape(NPT, 128, NPT, 128).transpose(2, 1, 0, 3)
        if split:
            return _split_bf(np.ascontiguousarray(tiles))
        return np.ascontiguousarray(tiles)

    wqh, wql = prep_w(Wq)
    wkh, wkl = prep_w(Wk)
    wvh, wvl = prep_w(Wv)
    wo = _r11(prep_w(Wo, split=False))
    wbt = np.ascontiguousarray(Wbeta.T).reshape(NPT, 128, H).transpose(1, 0, 2)
    wbh, wbl = _split_bf(np.ascontiguousarray(wbt))
    conv = np.stack([
        conv_q[:, 0, :].reshape(NPT, 128, KS),
        conv_k[:, 0, :].reshape(NPT, 128, KS),
        conv_v[:, 0, :].reshape(NPT, 128, KS),
    ]).transpose(2, 0, 1, 3)                                  # (128, 3, NPT, KS)
    conv = np.ascontiguousarray(conv, np.float32)
    rmsw = np.ascontiguousarray(rms_w.reshape(NPT, 128).T, np.float32)

    in_maps = []
    for r in range(NCORES):
        lo_pos = LC * r
        xs = x[:, lo_pos:lo_pos + LC, :]
        xT = np.ascontiguousarray(xs.transpose(0, 2, 1))      # (B, HID, LC)
        xT = xT.reshape(B, NPT, 128, LC).transpose(0, 2, 1, 3)  # (B,128,NPT,LC)
        xh, xl = _split_bf(np.ascontiguousarray(xT))
        if r == 0:
            xhalo = np.zeros((B, KS - 1, HID), np.float32)
        else:
            xhalo = x[:, lo_pos - (KS - 1):lo_pos, :]
        halo = np.stack([
            np.einsum("bth,oh->bot", xhalo, W).astype(np.float32)
            for W in (Wq, Wk, Wv)
        ])                                                    # (3, B, HID, 3)
        halo = np.ascontiguousarray(halo.reshape(3, B, NPT, 128, KS - 1))
        bm = np.zeros((128, NCORES), np.float32)
        bm[:, :r] = 1.0
        in_maps.append({
            "xh": xh, "xl": xl, "halo": halo,
            "wqh": wqh, "wql": wql, "wkh": wkh, "wkl": wkl,
            "wvh": wvh, "wvl": wvl, "wbh": wbh, "wbl": wbl, "wo": wo,
            "conv": conv, "rmsw": rmsw,
            "bmask": bm, "bimask": np.ascontiguousarray(1.0 - bm),
        })
    return in_maps


def kernel(**inputs):
    global _NC_CACHE
    in_maps = _build_in_maps(inputs)
    if _NC_CACHE is None:
        _NC_CACHE = build_nc()
    res = run_bass_kernel_spmd(_NC_CACHE, in_maps, core_ids=list(range(NCORES)))

    outs = []
    S = np.zeros((B, H, DH, DH), np.float32)
    for r in range(NCORES):
        om = res.results[r]
        outs.append(om["outT"].reshape(B, HID, LC).transpose(0, 2, 1))
        S += om["sp"]
    out = np.ascontiguousarray(np.concatenate(outs, axis=1), np.float32)
    return out, S
